# revision 97
# baseline (speedup 1.0000x reference)
"""Trainium2 Bass kernel for nn_Attention3d_9483287790337.

Math: 1x1x1-conv QKV -> per-head (softmax_d q * scale) @ (softmax_n k) attention
over n=4096 tokens -> out proj -> channel LayerNorm.

Key numerical fact exploited: k's softmax is over the 4096 tokens, so k-tilde
entries are ~2.4e-4 and sim = q~^T k~ lies in [0, ~1.6e-4]. exp(sim) is linear
to ~8 significant digits (error ~sim^2/2 ~ 1e-8 relative), so
  attn = softmax(sim) = (1 + sim) / (n + SCALE)      [denominator is constant:
                                                      sum_j sim_ij = SCALE]
and the attention output factorizes through associativity:
  out = (vsum + B^T q~s) / (n*(n+SCALE)),  B[d',d] = sum_j ek~T[j,d'] vT[j,d]
reducing the device work to ~n*d^2 instead of n^2*d.

Precision strategy (PE runs fp32 matmuls at 4 cycles/row, fp32r/bf16 at 1,
fp8 DoubleRow at 0.5): everything that only feeds the *correction* term
(q, k, ek, B, rsk -- all damped by sim ~1e-4 relative to vsum) runs in fp8
(projections via DoubleRow, ek storage, rsk column sums over jb-PAIRS) or
bf16; the precision-carrying v/vsum/out-projection path runs in fp32r (fp32
truncated to fp22, full PE rate).  Walrus requires fp32r operands to be
*produced* as fp32r, hence the f32r-typed tiles/DRAM tensors.  All ACT usage
stays on the one natural_log_exp table (1/x and 1/sqrt via exp(-ln)), so
there are zero ACT table reloads.  PSUM note: the interp zeroes a group's
whole (partitions x bank) region on start, so only same-partition-span
groups share a bank and the precision-carrying vsum group starts last.

Measured (CoreSim cost model, steady-state in-NEFF loop): 18.5us/iter vs
108us for the original fp32 kernel; device-validated rel_absmax 1.3e-3.

Sharding: 8-way data parallel over tokens (512 tokens/core); k/v/B work is
replicated per core (no cross-device comms). Weights replicated.  x arrives
host-rolled per core so its tokens are columns 0:TOK.
"""

import numpy as np
from contextlib import ExitStack

import concourse.bass as bass
import concourse.tile as tile
from concourse import mybir
import orjson

F32 = mybir.dt.float32
F32R = mybir.dt.float32r
BF16 = mybir.dt.bfloat16
FP8 = mybir.dt.float8e4
AX = mybir.AxisListType
OP = mybir.AluOpType
AF = mybir.ActivationFunctionType
DR = mybir.MatmulPerfMode.DoubleRow


DIM = 256
HEADS = 4
DHEAD = 64
N = 4096           # tokens
TOK = 512          # tokens per core
NCORES = 8
SCALE = DHEAD ** -0.5
NORM = 1.0 / (N * (N + SCALE))
NJB = N // 128     # 32 j-blocks

# --------------------------------------------------------------------------
# Workaround for this container's walrus build: its ISA encoding accepts at
# most ONE sync-wait per instruction, but tile.py emits several `on_wait`
# entries on one instruction. Split extras into single-wait NoOps on the same
# engine (engines execute their stream in order, so sequential waits are
# equivalent).
# --------------------------------------------------------------------------

_ENGINES = {"Pool", "Activation", "PE", "DVE", "SP"}
_SPLIT_OPCODE = "Drain"


def _split_multi_waits(bir_bytes: bytes) -> bytes:
    m = orjson.loads(bir_bytes)

    def walk(block):
        ins = block.get("instructions")
        if ins:
            out = []
            for inst in ins:
                si = inst.get("sync_info")
                waits = (si or {}).get("on_wait") or []
                if len(waits) > 1 and inst.get("engine") in _ENGINES:
                    for j, w in enumerate(waits[:-1]):
                        out.append({
                            "engine": inst["engine"],
                            "ins": [],
                            "outs": [],
                            "name": f"{inst.get('name', 'i')}_sw{j}",
                            "opcode": _SPLIT_OPCODE,
                            "sync_info": {"on_update": [], "on_wait": [w]},
                        })
                    si["on_wait"] = [waits[-1]]
                out.append(inst)
            block["instructions"] = out
        for sub in block.get("blocks") or []:
            walk(sub)

    for fn in m["functions"]:
        for b in fn["blocks"]:
            walk(b)
    return orjson.dumps(m)


_fix_installed = False


def _install_bir_fix():
    global _fix_installed
    if _fix_installed:
        return
    _fix_installed = True
    import concourse.bass_utils as bu
    import concourse.bass2jax as b2j

    orig = bu.compile_bir_kernel

    def patched(bir_json, tmpdir, neff_name="file.neff"):
        return orig(_split_multi_waits(bir_json), tmpdir, neff_name=neff_name)

    bu.compile_bir_kernel = patched
    b2j.compile_bir_kernel = patched


# --------------------------------------------------------------------------
# Device kernel
# --------------------------------------------------------------------------

def _make_pools(tc, ctx):
    const = ctx.enter_context(tc.tile_pool(name="const", bufs=1))
    # per-iteration tiles: bufs=2 rings give cross-iteration double buffering
    sb = ctx.enter_context(tc.tile_pool(name="sb", bufs=2))
    wk = ctx.enter_context(tc.tile_pool(name="wk", bufs=8))
    pp = ctx.enter_context(tc.tile_pool(name="pp", bufs=4, space="PSUM"))
    pacc = ctx.enter_context(tc.tile_pool(name="pacc", bufs=2, space="PSUM"))
    return const, sb, wk, pp, pacc


def _emit(nc, tc, ctx, t, pools=None, first=True, prev_tail=None):
    """Emit one iteration.  Returns a list of tail closures (numer / y path
    etc.) which the CALLER either injects into the next iteration's kv loop
    (steady-state pipelining) or emits directly (last / only iteration)."""
    if pools is None:
        pools = _make_pools(tc, ctx)
    const, sb, wk, pp, pacc = pools

    # previous iteration's rskv reshape first: its prskv psum slot must
    # recycle before this iteration's prskv allocation below
    if prev_tail:
        prev_tail[0]()

    # ---- true constants: emit once, shared by all iterations
    if first:
        ones_bf = const.tile([128, 1], BF16, name="ones_bf", tag="ones_bf")
        nc.vector.memset(ones_bf, 1.0)
        ones_col2 = const.tile([128, 2], BF16, name="ones_col2", tag="ones_col2")
        nc.vector.memset(ones_col2, 0.0)
        nc.vector.memset(ones_col2[0:64, 0:1], 1.0)
        nc.vector.memset(ones_col2[64:128, 1:2], 1.0)
        eps_t = const.tile([128, 1], F32, name="eps_t", tag="eps_t")
        nc.vector.memset(eps_t, 1e-5)
        ones8w = const.tile([128, 2, 128], FP8, name="ones8w", tag="ones8w")
        nc.vector.memset(ones8w, 1.0)
        ones_bfw = const.tile([128, 128], BF16, name="ones_bfw",
                              tag="ones_bfw")
        nc.vector.memset(ones_bfw, 1.0)
        # f32r all-ones for the vsum matmul (walrus rejects mixed 32/16-bit
        # matmul inputs; memset cannot emit f32r, a DVE copy can)
        ones_rw = const.tile([128, 128], F32R, name="ones_rw", tag="ones_rw")
        nc.vector.tensor_copy(ones_rw, ones_bfw)
        # ind2[h, d'] = SCALE where head-half h owns channel d' (else 0);
        # constant input, loaded once (partition-1-only memsets are illegal)
        ind2_sb = const.tile([2, 128], BF16, name="ind2_sb", tag="ind2_sb")
        nc.sync.dma_start(out=ind2_sb, in_=t["ind2"][:, :])
        _emit.consts = (ones_bf, ones_col2, eps_t, ind2_sb, ones8w, ones_bfw, ones_rw)
    ones_bf, ones_col2, eps_t, ind2_sb, ones8w, ones_bfw, ones_rw = _emit.consts

    # ---- inputs to SBUF.  x arrives host-rolled so this core's 512 tokens
    # are columns 0:TOK; weights first so the q path can start early.
    wv_sb, wo_sb = [], []
    for b in range(2):
        for nm, lst, drt, wdt, wdtype in (("wv", wv_sb, "wvT", 256, F32R),
                                          ("wo", wo_sb, "woT", 256, F32R)):
            w = sb.tile([128, wdt], wdtype, name=f"{nm}{b}", tag=f"{nm}{b}")
            nc.sync.dma_start(out=w, in_=t[drt][b * 128:(b + 1) * 128, :])
            lst.append(w)
    g_bc = sb.tile([128, 256], F32, name="g_bc", tag="g_bc")
    b_bc = sb.tile([128, 256], F32, name="b_bc", tag="b_bc")
    for dst, src in ((g_bc, t["g"]), (b_bc, t["bo"])):
        ap = src[:]
        bcast = bass.AP(tensor=ap.tensor, offset=ap.offset,
                        ap=[[0, 128]] + list(ap.ap))
        nc.gpsimd.dma_start(out=dst, in_=bcast)
    # fp8 copies for the q/k projections (correction path only, so ~6% fp8
    # noise is harmless): x8[ki, ko, j] = x[ki+128*ko, j] and
    # w{k,q}8[ki, ko, n] = w.T[ki+128*ko, n] -- DoubleRow reduction pairs.
    # DMA order: wq8/x8 (q path) and x chunk 0 first, then the rest --
    # keeps the single-shot lead-in short.
    wk8_sb = sb.tile([128, 2, 256], FP8, name="wk8", tag="wk8")
    nc.sync.dma_start(out=wk8_sb, in_=t["wk8"][:, :])
    wq8_sb = sb.tile([128, 2, 256], FP8, name="wq8", tag="wq8")
    nc.sync.dma_start(out=wq8_sb, in_=t["wq8"][:, :])
    x8_sb = sb.tile([128, 2, N], FP8, name="x8", tag="x8", bufs=2)
    for ko in range(2):
        nc.sync.dma_start(out=x8_sb[:, ko, :],
                          in_=t["x8"][:, ko * N:(ko + 1) * N])
    x_sb = [sb.tile([128, N], F32R, name=f"x{b}", tag=f"x{b}", bufs=2)
            for b in range(2)]
    for q in range(4):
        for b in range(2):
            nc.sync.dma_start(out=x_sb[b][:, q * 1024:(q + 1) * 1024],
                              in_=t["xf"][b * 128:(b + 1) * 128,
                                          q * 1024:(q + 1) * 1024])

    # ---- q path: q = w_q @ x[:, 0:TOK] as one fp8 DoubleRow matmul per
    # head-pair (q is correction-only); eq = exp(q).  The row-sum /
    # reciprocal chain (prs -> Ln -> rcp2) is deferred into the kv loop so
    # the PE never stalls on the ACT round trips.
    eq_sb, rcp2_sb = [], [None, None]
    for cb in range(2):
        pq = pp.tile([128, TOK], F32, name=f"pq{cb}", tag="p")
        nc.tensor.matmul(pq, wq8_sb[:, :, cb * 128:(cb + 1) * 128],
                         x8_sb[:, :, 0:TOK], start=True, stop=True,
                         perf_mode=DR)
        eq = sb.tile([128, TOK], BF16, name=f"eq{cb}", tag=f"eq{cb}")
        nc.scalar.activation(out=eq, in_=pq, func=AF.Exp)
        eq_sb.append(eq)

    qs_sb = [None, None]

    def q_rowsums(cb):
        # per-head-half column sums over the 128 partitions -> [2, TOK].
        # 1/x as exp(-ln(x)): Ln and Exp share one ACT table
        # (natural_log_exp_and_others), so the whole kernel runs without a
        # single ACT table reload.
        prs = pp.tile([2, TOK], F32, name=f"prs{cb}", tag="ps", bufs=2)
        nc.tensor.matmul(prs, ones_col2, eq_sb[cb], start=True,
                         stop=True)
        lnp = sb.tile([2, TOK], F32, name=f"lnp{cb}", tag=f"lnp{cb}")
        nc.scalar.activation(out=lnp, in_=prs, func=AF.Ln)
        rcp2 = sb.tile([2, TOK], BF16, name=f"rcp2{cb}", tag=f"rcp2{cb}")
        nc.scalar.activation(out=rcp2, in_=lnp, func=AF.Exp, scale=-1.0)
        rcp2_sb[cb] = rcp2

    def q_scale(cb):
        # q~s (sans rk, which folds into B): qs = eq * (SCALE * rcp2[half, i])
        qsc = pp.tile([128, TOK], F32, name=f"qsc{cb}", tag="p")
        nc.tensor.matmul(qsc, ind2_sb, rcp2_sb[cb], start=True,
                         stop=True)
        qs = sb.tile([128, TOK], BF16, name=f"qs{cb}", tag=f"qs{cb}")
        nc.vector.tensor_mul(out=qs, in0=eq_sb[cb], in1=qsc)
        qs_sb[cb] = qs

    # ---- fused kv path: ekT = exp(x^T w_k^T) stored fp8 (correction path),
    # vT = x^T w_v^T stored bf16 (vsum precision).  The j-contracted B
    # matmuls run mixed fp8-lhsT x bf16-rhs at 1 cycle/row; the rsk column
    # sums run as fp8 DoubleRow over jb-PAIRS (wide all-ones stationary, all
    # output rows identical, row 0 read) at 0.5 cycles/row; the vsum column
    # sums stay a bf16 ones-matmul.  rsk lands in combo[:, 0:256] (row 0)
    # and vsum in combo[0:1, 256:512], so the downstream [1, 512] extract is
    # unchanged.  B accumulates in-loop (pipelined one chunk behind).
    kk8 = sb.tile([128, NJB, 256], FP8, name="kk8", tag="kk8", bufs=1)
    vv = sb.tile([128, NJB, 256], BF16, name="vv", tag="vv", bufs=1)
    # jb-pair partial sums of v, built on the idle Pool engine: halves the
    # PE rows the vsum ones-matmul has to stream (sum over tokens is
    # associative; corresponding partitions of two j-blocks add first)
    vvp = sb.tile([128, NJB // 2, 256], F32R, name="vvp", tag="vvp", bufs=1)
    # rsk8 gets its own bank (a DoubleRow start corrupts co-resident
    # regular accumulation groups in the interp); B-cb0/B-cb1/vsum share
    # one bank as three interleaved regular groups (proven pattern).
    rsk8 = pacc.tile([128, 256], F32, name="rsk8", tag="acc")
    pBv = pacc.tile([128, 512], F32, name="pBv", tag="acc")

    def kv_consume(c0):
        # Within the shared pBv bank, vsum's group start must come LAST
        # (the interp zeroes the group's partition-span x whole bank on
        # start; HW is per-element, and only the B groups -- correction
        # path -- can tolerate the sim-only wipe).  After consume(0) no
        # group starts remain, so later consumes order copy-dependent
        # matmuls (vsum) before exp-dependent ones (B, rsk) to dodge
        # cross-engine latency.
        def mm_vsum():
            nc.tensor.matmul(pBv[:, 256:512], ones_rw, vvp[:, c0 // 2, :],
                             start=(c0 == 0), stop=(c0 == NJB - 2),
                             skip_group_check=True)

        def mm_B():
            for cb in range(2):
                for ci in range(2):
                    jb = c0 + ci
                    nc.tensor.matmul(pBv[:, 128 * cb:128 * cb + 128],
                                     kk8[:, jb, 128 * cb:128 * cb + 128],
                                     vv[:, jb, 128 * cb:128 * cb + 128],
                                     start=(jb == 0), stop=(jb == NJB - 1),
                                     skip_group_check=True)

        def mm_rsk():
            nc.tensor.matmul(rsk8, ones8w, kk8[:, c0:c0 + 2, :],
                             start=(c0 == 0), stop=(c0 == NJB - 2),
                             perf_mode=DR)

        if c0 == 0:
            mm_rsk(); mm_B(); mm_vsum()
        else:
            mm_vsum(); mm_B(); mm_rsk()

    inject = {2: lambda: q_rowsums(0), 4: lambda: q_rowsums(1),
              6: lambda: q_scale(0), 8: lambda: q_scale(1)}
    if prev_tail:
        for c0, seg in zip((6, 8, 10, 12, 14, 16, 18), prev_tail[1:]):
            prev = inject.get(c0)
            inject[c0] = (lambda p, s: (lambda: (p() if p else None, s())))(
                prev, seg)

    for c0 in range(0, NJB, 2):
        pk = pp.tile([128, 2, 256], F32, name=f"pk{c0}", tag="p")
        pv = pp.tile([128, 2, 256], F32, name=f"pv{c0}", tag="p")
        for ci in range(2):
            jb = c0 + ci
            # k: one fp8 DoubleRow matmul contracts all 256 input chans
            nc.tensor.matmul(pk[:, ci, :],
                             x8_sb[:, :, jb * 128:(jb + 1) * 128], wk8_sb,
                             start=True, stop=True, perf_mode=DR)
            for inb in range(2):
                nc.tensor.matmul(pv[:, ci, :],
                                 x_sb[inb][:, jb * 128:(jb + 1) * 128],
                                 wv_sb[inb], start=(inb == 0),
                                 stop=(inb == 1))
        if c0 >= 2:
            kv_consume(c0 - 2)
        if c0 in inject:
            inject[c0]()
        # exp on ACT only (no copies -> no act-table thrash); v copy
        # alternates DVE / ACT (Identity is in every ACT table) to balance
        nc.scalar.activation(out=kk8[:, c0:c0 + 2, :], in_=pk, func=AF.Exp)
        if (c0 // 2) % 4 == 3:
            nc.scalar.copy(out=vv[:, c0:c0 + 2, :], in_=pv)
        else:
            nc.vector.tensor_copy(vv[:, c0:c0 + 2, :], pv)
        nc.gpsimd.tensor_add(out=vvp[:, c0 // 2, :], in0=vv[:, c0, :],
                             in1=vv[:, c0 + 1, :])
    kv_consume(NJB - 2)

    # ---- tail segments (emitted inside the NEXT iteration's kv loop when
    # pipelining, so the PE never waits on the rskv reshape round trip)
    env = {}

    def seg_rskv():
        # rsk/vsum -> partition-major [128, 2] each via an SBUF round trip
        # (DMA reshapes across partitions; engine lanes cannot), then 1/rsk.
        rskvT = sb.tile([1, 512], F32, name="rskvT", tag="rskvT")
        nc.vector.tensor_copy(rskvT[0:1, 0:256], rsk8[0:1, :])
        nc.vector.tensor_copy(rskvT[0:1, 256:512], pBv[0:1, 256:512])
        rk_pre = sb.tile([128, 2], F32, name="rk_pre", tag="rk_pre")
        vsum_sb = sb.tile([128, 2], F32, name="vsum_sb", tag="vsum_sb")
        for cb in range(2):
            nc.gpsimd.dma_start(out=rk_pre[:, cb:cb + 1],
                                in_=rskvT[0:1, cb * 128:(cb + 1) * 128])
            nc.gpsimd.dma_start(
                out=vsum_sb[:, cb:cb + 1],
                in_=rskvT[0:1, 256 + cb * 128:256 + (cb + 1) * 128])
        rk_p = sb.tile([128, 2], F32, name="rk_p", tag="rk_p")
        nc.vector.reciprocal(rk_p, rk_pre)
        env["rk_p"], env["vsum"] = rk_p, vsum_sb

    def seg_B():
        # B psum -> sbuf diagonal head blocks, with rk[d'] folded in
        # (rk is diagonal in d', so it can scale B rows instead of qs)
        B_sb = sb.tile([128, 128], BF16, name="B_sb", tag="B_sb")
        for cb in range(2):
            for hp in range(2):
                r = 64 * hp
                nc.vector.tensor_scalar_mul(
                    out=B_sb[r:r + 64, 64 * cb:64 * cb + 64],
                    in0=pBv[r:r + 64, 128 * cb + r:128 * cb + r + 64],
                    scalar1=env["rk_p"][r:r + 64, cb:cb + 1])
        env["B_sb"] = B_sb
        env["out"] = [None, None]

    def seg_num(cb):
        # numer = B^T q~s; heads of a pair in partition halves
        B_sb = env["B_sb"]
        pnum = pp.tile([128, TOK], F32, name=f"pnum{cb}", tag="ps", bufs=2)
        for hp in range(2):
            row = 64 * hp
            nc.tensor.matmul(pnum[row:row + 64, :],
                             B_sb[row:row + 64, 64 * cb:64 * cb + 64],
                             qs_sb[cb][row:row + 64, :],
                             start=True, stop=True)
        oa = sb.tile([128, TOK], F32R, name=f"oall{cb}", tag=f"oall{cb}")
        nc.vector.tensor_scalar(out=oa, in0=pnum,
                                scalar1=env["vsum"][:, cb:cb + 1],
                                scalar2=NORM, op0=OP.add, op1=OP.mult)
        env["out"][cb] = oa

    def seg_y(tb):
        # y = w_out @ out + b_out (as yT [t, o]), then channel LayerNorm
        out_all = env["out"]
        py = pp.tile([128, 256], F32, name=f"py{tb}", tag="ps", bufs=2)
        for cb in range(2):
            nc.tensor.matmul(py, out_all[cb][:, tb * 128:(tb + 1) * 128],
                             wo_sb[cb], start=(cb == 0), stop=(cb == 1))
        yb = wk.tile([128, 256], F32, name=f"yb{tb}", tag="w_yb")
        nc.vector.tensor_add(out=yb, in0=py, in1=b_bc)
        stats = wk.tile([128, 6], F32, name=f"st{tb}", tag="w_small")
        nc.vector.bn_stats(out=stats, in_=yb)
        mv = wk.tile([128, 2], F32, name=f"mv{tb}", tag="w_small")
        nc.vector.bn_aggr(out=mv, in_=stats)
        # rstd = exp(-0.5 ln(var + eps)) -- stays on the Ln/Exp ACT table
        lnv = wk.tile([128, 1], F32, name=f"lv{tb}", tag="w_small")
        nc.scalar.activation(out=lnv, in_=mv[:, 1:2], func=AF.Ln,
                             bias=eps_t, scale=1.0)
        rstd = wk.tile([128, 1], F32, name=f"rs{tb}", tag="w_small")
        nc.scalar.activation(out=rstd, in_=lnv, func=AF.Exp, scale=-0.5)
        yn = wk.tile([128, 256], F32, name=f"yn{tb}", tag="w_yb")
        nc.vector.tensor_scalar(out=yn, in0=yb, scalar1=mv[:, 0:1],
                                scalar2=rstd, op0=OP.subtract, op1=OP.mult)
        yo = wk.tile([128, 256], F32, name=f"yo{tb}", tag="w_yb")
        nc.gpsimd.tensor_mul(out=yo, in0=yn, in1=g_bc)
        nc.gpsimd.dma_start(out=t["yt"][tb * 128:(tb + 1) * 128, :], in_=yo)

    return [seg_rskv, seg_B, lambda: seg_num(0), lambda: seg_num(1),
            lambda: seg_y(0), lambda: seg_y(1), lambda: seg_y(2),
            lambda: seg_y(3)]


def build_nc(niter=1):
    nc = bass.Bass()
    t = {
        "xf": nc.dram_tensor("xf", [DIM, N], F32R, kind="ExternalInput"),
        "wvT": nc.dram_tensor("wvT", [DIM, DIM], F32R, kind="ExternalInput"),
        "woT": nc.dram_tensor("woT", [DIM, DIM], F32R, kind="ExternalInput"),
        "g": nc.dram_tensor("g", [DIM], F32, kind="ExternalInput"),
        "bo": nc.dram_tensor("bo", [DIM], F32, kind="ExternalInput"),
        "ind2": nc.dram_tensor("ind2", [2, 128], BF16, kind="ExternalInput"),
        "x8": nc.dram_tensor("x8", [128, 2 * N], FP8, kind="ExternalInput"),
        "wk8": nc.dram_tensor("wk8", [128, 512], FP8, kind="ExternalInput"),
        "wq8": nc.dram_tensor("wq8", [128, 512], FP8, kind="ExternalInput"),
        "yt": nc.dram_tensor("yt", [TOK, DIM], F32, kind="ExternalOutput"),
    }
    with tile.TileContext(nc) as tc:
        with ExitStack() as ctx:
            pools = _make_pools(tc, ctx)
            tail = None
            for it in range(niter):
                tail = _emit(nc, tc, ctx, t, pools, first=(it == 0),
                             prev_tail=tail)
            for seg in tail:
                seg()
    return nc


_NC_CACHE = {}


def _make_ind2():
    import ml_dtypes
    ind2 = np.zeros((2, 128), ml_dtypes.bfloat16)
    ind2[0, 0:64] = SCALE
    ind2[1, 64:128] = SCALE
    return ind2


def _prep_inputs(x, w_qkv, w_out, b_out, g):
    import ml_dtypes
    xf = np.ascontiguousarray(x.reshape(DIM, N).astype(np.float32))
    w_q, w_k, w_v = (w_qkv[0:256], w_qkv[256:512], w_qkv[512:768])
    def _dr_pack_w(w):
        return np.ascontiguousarray(
            np.asarray(w.T).astype(ml_dtypes.float8_e4m3)
            .reshape(2, 128, 256).transpose(1, 0, 2).reshape(128, 512))
    wk8 = _dr_pack_w(w_k)
    wq8 = _dr_pack_w(w_q)
    common = {
        "wvT": np.ascontiguousarray(w_v.T.astype(np.float32)),
        "woT": np.ascontiguousarray(w_out.T.astype(np.float32)),
        "g": np.ascontiguousarray(g.astype(np.float32)),
        "bo": np.ascontiguousarray(b_out.astype(np.float32)),
        "ind2": _make_ind2(),
        "wk8": wk8,
        "wq8": wq8,
    }
    in_maps = []
    for c in range(NCORES):
        m = dict(common)
        # roll so this core's TOK tokens are columns 0:TOK (j-sums are
        # permutation-invariant, so k/v/B math is unaffected)
        xr = np.ascontiguousarray(np.roll(xf, -c * TOK, axis=1))
        m["xf"] = xr
        # DoubleRow-packed fp8 copy: x8[ki, ko, j] = x[ki+128*ko, j]
        m["x8"] = np.ascontiguousarray(
            xr.astype(ml_dtypes.float8_e4m3)
            .reshape(2, 128, N).transpose(1, 0, 2).reshape(128, 2 * N))
        in_maps.append(m)
    return in_maps


def kernel(x, w_qkv, w_out, b_out, g):
    _install_bir_fix()
    from concourse.bass_utils import run_bass_kernel_spmd

    if "nc" not in _NC_CACHE:
        _NC_CACHE["nc"] = build_nc()
    nc = _NC_CACHE["nc"]
    in_maps = _prep_inputs(np.asarray(x), np.asarray(w_qkv), np.asarray(w_out),
                           np.asarray(b_out), np.asarray(g))
    res = run_bass_kernel_spmd(nc, in_maps, core_ids=list(range(NCORES)))
    y = np.empty((DIM, N), np.float32)
    for c in range(NCORES):
        y[:, c * TOK:(c + 1) * TOK] = res.results[c]["yt"].T
    return y.reshape(1, DIM, 16, 16, 16)


if __name__ == "__main__":
    import reference as R
    inputs = {k: np.asarray(v) for k, v in R.setup_inputs().items()}
    ref = np.asarray(R.reference(**inputs))
    got = kernel(**inputs)
    err = np.abs(got - ref)
    print("rel_absmax:", err.max() / np.abs(ref).max())

